# revision 19
# baseline (speedup 1.0000x reference)
"""Trainium2 Bass kernel for a 12-head attention block with post-softmax
additive per-head bias.

    qkv = x @ W_qkv                          x: [64, 196, 768]
    attn = softmax(q k^T / 8) + static_a     (bias added AFTER softmax)
    out = (attn @ v) @ W_proj + b_proj

Sharding: data-parallel over batch across 8 NeuronCores (8 batches each).
No collectives. Weights replicated; x passed transposed per batch so the
contraction dim lands on SBUF partitions.

Precision split: the softmax path tolerates fp8 because the final output
is dominated by the static_a @ v term (||a@v|| ~ 12 vs ||softmax@v|| ~
0.2), so q/k and P=exp(S) run in fp8 (DoubleRow matmuls, 2x PE rate)
while v, static_a@v and the projection stay bf16.

Per-core dataflow, software-pipelined over the 8 local batches b:
  qk(b)  : q^T,k^T = W_qk^T @ x^T   fp8 DoubleRow (K=256/group), f32 PSUM,
           copies to bf16 SBUF alternate DVE/ACT, odd heads staged to
           base partition 0 by SBUF-SBUF DMA (64-row matmul at base 64
           after one at base 0 crashes HW).
  v(b)   : v = x @ W_v              bf16; PSUM copied to bf16 vb (ACT)
           and re-quantized to fp8 v8 with a ones column (Pool engine).
  uo(b-1): U = P^T.T @ [v|1]        fp8 DoubleRow over both m-chunks at
           once (ones column gives softmax row sums r); then on DVE:
           1/r, U*(1/r), + static AV read straight from PSUM -> ob.
  tr(b-1): O^T via PE transpose -> aot (proj lhsT layout).
  st_av(b): S^T = k q^T (bf16) -> exp(S/8 - 2) on ACT -> fp8 P^T
           (the -2 keeps exp below the TRN e4m3 max of 240; softmax is
           shift-invariant). A@v (bf16) interleaved between exp-paced
           S^T PSUM groups to keep the PE array busy.
  proj   : out = aot.T @ W_proj + b_proj   bf16, bias via DVE add.

P^T and v8 live in ping-pong persistent tiles: partitions 68-127 of the
second m-chunk are zeroed once at init so the DoubleRow contraction over
m=196 (padded to 2x128) is exact.
"""

import os
import sys

_TRN_REPO = "/opt/trn_rl_repo"
if _TRN_REPO not in sys.path:
    sys.path.insert(0, _TRN_REPO)

import numpy as np
import ml_dtypes

import concourse.bass as bass
import concourse.tile as tile
from concourse import bacc, mybir
from concourse.bass import MemorySpace
from concourse.bass_utils import run_bass_kernel_spmd
from concourse.masks import make_identity

BF16 = mybir.dt.bfloat16
F32 = mybir.dt.float32
FP8 = mybir.dt.float8e4

N_CORES = 8
BATCH = 64
B = BATCH // N_CORES  # 8 local batches per core
H = 12
D = 64
N = 196
C = 768
T = B * N  # 1568 local tokens
KC = 6  # contraction chunks of 128 over C=768
G = 3  # DoubleRow contraction groups of 256 over C=768
SCALE = D ** -0.5  # 0.125
ESHIFT = -2.0  # exp(S/8 - 2): keeps P under the TRN e4m3 max (240)

# token chunks of 128 over T (for the projection)
MCS = [(i * 128, min(128, T - i * 128)) for i in range((T + 127) // 128)]
# per-batch row chunks over N=196
NCH = [(0, 128), (128, 68)]

AluOp = mybir.AluOpType
ActFn = mybir.ActivationFunctionType
PM = mybir.MatmulPerfMode


def _emit(nc: bass.Bass):
    # xt: per-batch partition-major x^T blocks: xt[b, p, kc*N+n] = x[b, n, kc*128+p]
    # x8: same layout in fp8 (for the q/k DoubleRow GEMM)
    # at: partition-major A^T: at[mc, p, h*N+n] = A[h, n, mc*128+p]
    xt_d = nc.declare_dram_parameter("xt", [B, 128, KC * N], BF16, isOutput=False)
    x8_d = nc.declare_dram_parameter("x8", [B, 128, KC * N], FP8, isOutput=False)
    wqk8_d = nc.declare_dram_parameter("wqk8", [128, KC * 2 * C], FP8, isOutput=False)
    wv_d = nc.declare_dram_parameter("wv", [128, KC * C], BF16, isOutput=False)
    at_d = nc.declare_dram_parameter("at", [2, 128, H * N], BF16, isOutput=False)
    wproj_d = nc.declare_dram_parameter("wproj", [C, C], BF16, isOutput=False)
    bproj_d = nc.declare_dram_parameter("bproj", [1, C], BF16, isOutput=False)
    out_d = nc.declare_dram_parameter("out", [T, C], F32, isOutput=True)

    with tile.TileContext(nc) as tc:
        from contextlib import ExitStack

        with ExitStack() as stk:
            const = stk.enter_context(tc.tile_pool(name="const", bufs=1))
            wq = stk.enter_context(tc.tile_pool(name="wq", bufs=1))
            xtp = stk.enter_context(tc.tile_pool(name="xtp", bufs=4))
            x8p = stk.enter_context(tc.tile_pool(name="x8p", bufs=4))
            qkp = stk.enter_context(tc.tile_pool(name="qkp", bufs=4))
            vbp = stk.enter_context(tc.tile_pool(name="vbp", bufs=4))
            obp = stk.enter_context(tc.tile_pool(name="obp", bufs=4))
            persist = stk.enter_context(tc.tile_pool(name="persist", bufs=1))
            stage = stk.enter_context(tc.tile_pool(name="stage", bufs=3))
            small = stk.enter_context(tc.tile_pool(name="small", bufs=3))
            outst = stk.enter_context(tc.tile_pool(name="outst", bufs=4))
            aotp = stk.enter_context(tc.tile_pool(name="aotp", bufs=1))

            # ---- constants (cheap; loaded early) ----
            ident = const.tile([128, 128], BF16)
            make_identity(nc, ident)
            cbias = const.tile([128, 1], F32)
            nc.vector.memset(cbias, ESHIFT)

            wqk8_sb = wq.tile([128, KC, 2 * C], FP8)
            wv_sb = wq.tile([128, KC, C], BF16)
            at_sb = const.tile([128, 2, H * N], BF16)
            wproj_sb = const.tile([128, KC, C], BF16)
            bias_bc = const.tile([128, C], BF16)
            aot_sb = aotp.tile([128, KC, T], BF16)

            # ping-pong persistent tiles: P^T (fp8) and v|1 (fp8), with the
            # m-chunk index as a free dim so one DoubleRow matmul contracts
            # all of m=196 (partitions 68-127 of chunk 1 stay zero).
            pt_pp = [
                persist.tile([128, 2, H, N], FP8, name=f"pt{i}", tag=f"pt{i}")
                for i in range(2)
            ]
            v8_pp = [
                persist.tile([128, 2, H, 65], FP8, name=f"v8{i}", tag=f"v8{i}")
                for i in range(2)
            ]
            def emit_pad_memsets():
                # engines need 0/32/64/96-aligned start partitions: zero
                # from 64 (64-67 get valid data every batch).  Emitted after
                # st_av(0) — only uo(0), in the b=1 section, reads the pads,
                # and the DVE is idle during st_av.
                for ptile in pt_pp:
                    nc.vector.memset(ptile[64:128, 1, :, :], 0.0)
                for vtile in v8_pp:
                    nc.vector.memset(vtile[64:128, 1, :, :], 0.0)
                    nc.vector.memset(vtile[0:128, 0, :, 64:65], 1.0)
                    nc.vector.memset(vtile[0:64, 1, :, 64:65], 1.0)
                    nc.vector.memset(vtile[64:68, 1, :, 64:65], 1.0)

            # per-batch rotating tiles, tracked across loop iterations
            xtb_t = {}
            x8b_t = {}
            qtb_t = {}
            ktb_t = {}
            vb_t = {}
            ob_t = {}

            def emit_load_x(b):
                xtb = xtp.tile([128, KC, N], BF16, tag="xtb")
                x8b = x8p.tile([128, KC, N], FP8, tag="x8b")
                xtb_t[b], x8b_t[b] = xtb, x8b
                nc.sync.dma_start(out=xtb.rearrange("p k n -> p (k n)"), in_=xt_d[b])
                # Pool-issued DMA: keeps the SP sequencer free (a DMA holds
                # its issuing engine's SEQ for the whole transfer)
                nc.gpsimd.dma_start(
                    out=x8b.rearrange("p k n -> p (k n)"), in_=x8_d[b]
                )

            def emit_qk(b):
                # q^T, k^T via fp8 DoubleRow: lhsT = W slice [128, 2, 128]
                # (256-deep contraction per group), rhs = x^T [128, 2, 196].
                x8b = x8b_t[b]
                qtb = qkp.tile([128, KC, N], BF16, tag="qtb")
                ktb = qkp.tile([128, KC, N], BF16, tag="ktb")
                qtb_t[b], ktb_t[b] = qtb, ktb
                stq = stage.tile([64, KC, N], BF16, tag="stq")
                stk_ = stage.tile([64, KC, N], BF16, tag="stk")
                for si, (dst, sec, stg) in enumerate(
                    ((qtb, 0, stq), (ktb, C, stk_))
                ):
                    for cp in range(3):  # pairs of 128-col chunks
                        ps = psA.tile([128, 512], F32, tag="pA")
                        for cc in range(2):
                            c = cp * 2 + cc
                            for g in range(G):
                                nc.tensor.matmul(
                                    ps[:, cc * 256 : cc * 256 + N],
                                    lhsT=wqk8_sb[
                                        :, 2 * g : 2 * g + 2,
                                        sec + c * 128 : sec + (c + 1) * 128,
                                    ],
                                    rhs=x8b[:, 2 * g : 2 * g + 2, :],
                                    start=(g == 0),
                                    stop=(g == G - 1),
                                    perf_mode=PM.DoubleRow,
                                )
                        dstv = dst[:, cp * 2 : cp * 2 + 2, :]
                        srcv = ps.rearrange("p (c n) -> p c n", c=2)[:, :, 0:N]
                        # alternate engines so copies pace at ~2x
                        if (si * 3 + cp) % 2 == 0:
                            nc.vector.tensor_copy(dstv, srcv)
                        else:
                            nc.scalar.copy(dstv, srcv)
                    nc.gpsimd.dma_start(out=stg, in_=dst[64:128, :, :])
                return stq, stk_

            def emit_v_quarter(b, mc, ns):
                xtb = xtb_t[b]
                v8t = v8_pp[b % 2]
                if b not in vb_t:
                    vb_t[b] = vbp.tile([128, 2, H * D], BF16, name="vb", tag="vb")
                vb = vb_t[b]
                moff, mlen = NCH[mc]
                ps = psA.tile([128, 512], F32, tag="pA")
                for kc in range(KC):
                    nc.tensor.matmul(
                        ps[0:mlen, 0:384],
                        lhsT=xtb[:, kc, moff : moff + mlen],
                        rhs=wv_sb[:, kc, ns * 384 : (ns + 1) * 384],
                        start=(kc == 0),
                        stop=(kc == KC - 1),
                    )
                dstv = vb[0:mlen, mc, ns * 384 : (ns + 1) * 384]
                if mc == 1 and ns == 1:
                    nc.vector.tensor_copy(dstv, ps[0:mlen, 0:384])
                else:
                    nc.scalar.copy(dstv, ps[0:mlen, 0:384])
                # fp8 copy for the U matmul (Pool: SBUF->SBUF only)
                nc.gpsimd.tensor_copy(
                    v8t[0:mlen, mc, ns * 6 : (ns + 1) * 6, 0:64],
                    dstv.rearrange("p (h c) -> p h c", h=6),
                )

            av_t = {}

            def emit_st_av(b, stq, stk_):
                # Interleave the A@v matmul blocks between the exp-paced S^T
                # PSUM groups: PE executes in order, so a stalled S^T group
                # would otherwise idle the array while ACT drains exp.
                qtb, ktb = qtb_t[b], ktb_t[b]
                vb = vb_t[b]
                ptt = pt_pp[b % 2]

                def st_unit(mc, hg):
                    moff, mlen = NCH[mc]
                    ps = psB.tile([128, 1024], F32, tag="pB")
                    for hh in range(4):
                        h = hg * 4 + hh
                        off = (hh // 2) * 512 + (hh % 2) * 196
                        if h % 2 == 0:
                            lhsT = ktb[0:64, h // 2, moff : moff + mlen]
                            rhs = qtb[0:64, h // 2, :]
                        else:
                            lhsT = stk_[0:64, h // 2, moff : moff + mlen]
                            rhs = stq[0:64, h // 2, :]
                        nc.tensor.matmul(
                            ps[0:mlen, off : off + 196],
                            lhsT=lhsT,
                            rhs=rhs,
                            start=True,
                            stop=True,
                        )
                    src = ps.rearrange("p (k x) -> p k x", k=2)[
                        0:mlen, :, 0:392
                    ].rearrange("p k (h n) -> p k h n", h=2)
                    dst = ptt[0:mlen, mc, hg * 4 : (hg + 1) * 4, :].rearrange(
                        "p (k h) n -> p k h n", k=2
                    )
                    nc.scalar.activation(
                        dst, src, ActFn.Exp, bias=cbias[0:mlen, :], scale=SCALE
                    )

                av_tile = {}

                def av_unit(nc_i, hblk):
                    noff, nlen = NCH[nc_i]
                    if nc_i not in av_tile:
                        av = psB.tile([128, 1024], F32, tag="pB")
                        av_tile[nc_i] = av
                        av_t[(b, nc_i)] = av
                    av = av_tile[nc_i]
                    for h in range(hblk * 3, hblk * 3 + 3):
                        aoff = (h // 8) * 512 + (h % 8) * 64
                        for mc, (moff, mlen) in enumerate(NCH):
                            nc.tensor.matmul(
                                av[0:nlen, aoff : aoff + 64],
                                lhsT=at_sb[
                                    0:mlen, mc, h * N + noff : h * N + noff + nlen
                                ],
                                rhs=vb[0:mlen, mc, h * D : h * D + D],
                                start=(mc == 0),
                                stop=(mc == 1),
                            )

                order = [
                    ("st", 0, 0), ("av", 0, 0), ("st", 0, 1), ("av", 0, 1),
                    ("st", 0, 2), ("av", 0, 2), ("st", 1, 0), ("av", 0, 3),
                    ("st", 1, 1), ("av", 1, 0), ("st", 1, 2), ("av", 1, 1),
                    ("av", 1, 2), ("av", 1, 3),
                ]
                for kind, a, bb_ in order:
                    if kind == "st":
                        st_unit(a, bb_)
                    else:
                        av_unit(a, bb_)

            tmp_t = {}

            def emit_u_unit(b, nc_i, half):
                # U = P^T.T @ [v|1] via one fp8 DoubleRow matmul per head
                # (both m-chunks contracted at once); 65th column is the
                # softmax row sum r.  tmp = U * (1/r) on DVE.
                ptt = pt_pp[b % 2]
                v8t = v8_pp[b % 2]
                noff, nlen = NCH[nc_i]
                if (b, nc_i) not in tmp_t:
                    rec = small.tile([128, H], F32, tag="rec")
                    tmp = small.tile([128, C], F32, tag="tmp")
                    tmp_t[(b, nc_i)] = (rec, tmp)
                rec, tmp = tmp_t[(b, nc_i)]
                uph = psA.tile([128, 512], F32, tag="pA")
                for h in range(half * 6, half * 6 + 6):
                    uoff = (h % 6) * 65
                    nc.tensor.matmul(
                        uph[0:nlen, uoff : uoff + 65],
                        lhsT=ptt[0:128, 0:2, h, noff : noff + nlen],
                        rhs=v8t[0:128, 0:2, h, 0:65],
                        start=True,
                        stop=True,
                        perf_mode=PM.DoubleRow,
                    )
                upv = uph[0:nlen, 0:390].rearrange("p (h x) -> p h x", h=6)
                recv = rec[0:nlen, half * 6 : half * 6 + 6, None]
                nc.vector.reciprocal(recv, upv[:, :, 64:65])
                nc.vector.tensor_tensor(
                    tmp[0:nlen, half * 384 : (half + 1) * 384].rearrange(
                        "p (h c) -> p h c", h=6
                    ),
                    upv[:, :, 0:64],
                    recv.to_broadcast((nlen, 6, 64)),
                    AluOp.mult,
                )

            def emit_u_add(b, nc_i, half):
                # ob = tmp + AV, with AV read straight from its PSUM tile
                noff, nlen = NCH[nc_i]
                if b not in ob_t:
                    ob_t[b] = obp.tile([128, 2, C], BF16, name="ob", tag="ob")
                ob = ob_t[b]
                av = av_t[(b, nc_i)]
                _, tmp = tmp_t[(b, nc_i)]
                nc.vector.tensor_tensor(
                    ob[0:nlen, nc_i, half * 384 : (half + 1) * 384],
                    tmp[0:nlen, half * 384 : (half + 1) * 384],
                    av[0:nlen, half * 384 : (half + 1) * 384],
                    AluOp.add,
                )

            def emit_tr_pair(b, nc_i, hp):
                ob = ob_t[b]
                noff, nlen = NCH[nc_i]
                tp = psB.tile([128, 512], BF16, tag="pB")
                for j in range(2):
                    nc.tensor.transpose(
                        tp[:, j * 256 : j * 256 + nlen],
                        in_=ob[0:nlen, nc_i, (hp + j) * 128 : (hp + j + 1) * 128],
                        identity=ident[0:nlen, 0:nlen],
                    )
                nc.vector.tensor_copy(
                    aot_sb[:, hp : hp + 2, b * N + noff : b * N + noff + nlen],
                    tp.rearrange("p (j x) -> p j x", j=2)[:, :, 0:nlen],
                )

            def emit_proj_chunk(mc, pps, tag="pB"):
                moff, mlen = MCS[mc]
                pp = pps.tile([128, 1024], F32, tag=tag)
                for nsl, nw in ((0, 512), (512, 256)):
                    for kc in range(KC):
                        nc.tensor.matmul(
                            pp[0:mlen, nsl : nsl + nw],
                            lhsT=aot_sb[:, kc, moff : moff + mlen],
                            rhs=wproj_sb[:, kc, nsl : nsl + nw],
                            start=(kc == 0),
                            stop=(kc == KC - 1),
                        )
                ot = outst.tile([128, C], F32, tag="ot")
                nc.vector.tensor_tensor(
                    ot[0:mlen, :],
                    pp[0:mlen, 0:768],
                    bias_bc[0:mlen, :],
                    AluOp.add,
                )
                nc.sync.dma_start(
                    out=out_d[moff : moff + mlen, :], in_=ot[0:mlen, :]
                )

            with (
                tc.tile_pool(name="psA", bufs=2, space=MemorySpace.PSUM) as psA,
                tc.tile_pool(name="psB", bufs=3, space=MemorySpace.PSUM) as psB,
            ):
                # input DMAs for batch 0; weights spread across the SP/ACT/
                # Pool queues (a DMA occupies its issuing engine's sequencer
                # for the whole transfer, so one queue would serialize them)
                nc.sync.dma_start(
                    out=wqk8_sb[:, 0:3, :], in_=wqk8_d[:, 0 : 3 * 2 * C]
                )
                nc.scalar.dma_start(
                    out=wqk8_sb[:, 3:6, :], in_=wqk8_d[:, 3 * 2 * C : 6 * 2 * C]
                )
                emit_load_x(0)
                nc.scalar.dma_start(out=wv_sb[:, 0:3, :], in_=wv_d[:, 0 : 3 * C])
                nc.gpsimd.dma_start(
                    out=wv_sb[:, 3:6, :], in_=wv_d[:, 3 * C : 6 * C]
                )
                for mc in range(2):
                    nc.sync.dma_start(out=at_sb[:, mc, :], in_=at_d[mc])
                def emit_uo_tr(b):
                    # interleave U-normalize units with v quarters' psA
                    # rotation partners handled by caller; here: adds + tr
                    for nc_i in range(2):
                        emit_u_add(b, nc_i, 0)
                        emit_u_add(b, nc_i, 1)
                        for hp in range(0, KC, 2):
                            emit_tr_pair(b, nc_i, hp)

                # proj chunk i covers tokens [128i, 128(i+1)): ready once the
                # last batch it touches has been transposed into aot
                chunk_ready = {}
                for i in range(len(MCS)):
                    last_b = (min((i + 1) * 128, T) - 1) // N
                    chunk_ready.setdefault(last_b, []).append(i)

                stqk = {}
                for b in range(B):
                    stqk[b] = emit_qk(b)
                    if b + 1 < B:
                        emit_load_x(b + 1)
                    # v quarters interleaved with U units of the previous
                    # batch: alternating psA consumers (ACT/Pool vs DVE)
                    # so neither engine paces the pool rotation alone
                    quarters = [(0, 0), (0, 1), (1, 0), (1, 1)]
                    for qi, (mc, ns) in enumerate(quarters):
                        emit_v_quarter(b, mc, ns)
                        if b > 0:
                            emit_u_unit(b - 1, qi // 2, qi % 2)
                    if b > 0:
                        emit_uo_tr(b - 1)
                        # interleave ready projection chunks: PE filler that
                        # absorbs the uo/exp-paced engine chains
                        for i in chunk_ready.get(b - 1, []):
                            emit_proj_chunk(i, psB, tag="pB")
                    emit_st_av(b, *stqk[b])
                    if b == 0:
                        emit_pad_memsets()
                        # wproj in two 3-chunk DMAs (3D source AP)
                        for half in range(2):
                            wp_ap = bass.AP(
                                tensor=wproj_d.ap().tensor,
                                offset=half * 3 * 128 * C,
                                ap=[[C, 128], [128 * C, 3], [1, C]],
                            )
                            nc.sync.dma_start(
                                out=wproj_sb[:, half * 3 : half * 3 + 3, :],
                                in_=wp_ap,
                            )
                        bproj_ap = bass.AP(
                            tensor=bproj_d.ap().tensor,
                            offset=0,
                            ap=[[0, 128], [1, C]],
                        )
                        nc.gpsimd.dma_start(out=bias_bc, in_=bproj_ap)
                for qi in range(4):
                    emit_u_unit(B - 1, qi // 2, qi % 2)
                emit_uo_tr(B - 1)
                for i in chunk_ready.get(B - 1, []):
                    emit_proj_chunk(i, psB, tag="pB")

    return nc


_CACHE: dict = {}


def _get_module():
    if "nc" not in _CACHE:
        nc = bacc.Bacc(None, target_bir_lowering=False)
        _emit(nc)
        nc.compile()
        _CACHE["nc"] = nc
    return _CACHE["nc"]


def _prep_inputs(x, W_qkv, static_a, W_proj, b_proj):
    """Host-side shard + layout prep; returns per-core input maps."""
    bf = ml_dtypes.bfloat16
    f8 = ml_dtypes.float8_e4m3
    x = np.asarray(x, dtype=np.float32)
    Wf = np.asarray(W_qkv, dtype=np.float32)
    # kc-major weight views: [128, KC, cols]
    Wr = Wf.reshape(KC, 128, 3 * C)
    wqk8_b = np.ascontiguousarray(
        Wr[:, :, 0 : 2 * C].transpose(1, 0, 2)
    ).reshape(128, KC * 2 * C).astype(f8)
    wv_b = np.ascontiguousarray(
        Wr[:, :, 2 * C : 3 * C].transpose(1, 0, 2)
    ).reshape(128, KC * C).astype(bf)
    A = np.asarray(static_a, dtype=np.float32)[0]  # [H, N, N]
    Am = np.ascontiguousarray(A.transpose(2, 0, 1))  # [m, H, n]
    at_arr = np.zeros((2, 128, H, N), dtype=np.float32)
    at_arr[0] = Am[0:128]
    at_arr[1, 0:68] = Am[128:196]
    at_b = at_arr.reshape(2, 128, H * N).astype(bf)
    wproj_b = np.asarray(W_proj, dtype=np.float32).astype(bf)
    bproj_b = np.asarray(b_proj, dtype=np.float32).reshape(1, C).astype(bf)

    in_maps = []
    for i in range(N_CORES):
        shard = x[i * B : (i + 1) * B]  # [B, N, C]
        # [B, 128, KC*N]: xt[b, p, kc*N + n] = x[b, n, kc*128 + p]
        xt_f = np.ascontiguousarray(
            shard.transpose(0, 2, 1)
            .reshape(B, KC, 128, N)
            .transpose(0, 2, 1, 3)
            .reshape(B, 128, KC * N)
        )
        in_maps.append(
            dict(
                xt=xt_f.astype(bf),
                x8=xt_f.astype(f8),
                wqk8=wqk8_b,
                wv=wv_b,
                at=at_b,
                wproj=wproj_b,
                bproj=bproj_b,
            )
        )
    return in_maps


_last_results = None


def kernel(x, W_qkv, static_a, W_proj, b_proj):
    global _last_results
    in_maps = _prep_inputs(x, W_qkv, static_a, W_proj, b_proj)
    nc = _get_module()
    res = run_bass_kernel_spmd(nc, in_maps, core_ids=list(range(N_CORES)))
    _last_results = res
    out = np.concatenate(
        [np.asarray(r["out"]).reshape(B, N, C) for r in res.results], axis=0
    )
    return out.astype(np.float32)


# revision 24
# speedup vs baseline: 1.1038x; 1.1038x over previous
"""Trainium2 Bass kernel for a 12-head attention block with post-softmax
additive per-head bias.

    qkv = x @ W_qkv                          x: [64, 196, 768]
    attn = softmax(q k^T / 8) + static_a     (bias added AFTER softmax)
    out = (attn @ v) @ W_proj + b_proj

Sharding: data-parallel over batch across 8 NeuronCores (8 batches each).
No collectives. Weights replicated; x passed transposed per batch so the
contraction dim lands on SBUF partitions.

Precision split: the softmax path tolerates fp8 because the final output
is dominated by the static_a @ v term (||a@v|| ~ 12 vs ||softmax@v|| ~
0.2), so q/k and P=exp(S) run in fp8 (DoubleRow matmuls, 2x PE rate)
while v, static_a@v and the projection stay bf16.

Per-core dataflow, software-pipelined over the 8 local batches b:
  qk(b)  : q^T,k^T = W_qk^T @ x^T   fp8 DoubleRow (K=256/group), f32 PSUM,
           copies to bf16 SBUF alternate DVE/ACT, odd heads staged to
           base partition 0 by SBUF-SBUF DMA (64-row matmul at base 64
           after one at base 0 crashes HW).
  v(b)   : v = x @ W_v              bf16; PSUM copied to bf16 vb (ACT)
           and re-quantized to fp8 v8 with a ones column (Pool engine).
  uo(b-1): U = P^T.T @ [v|1]        fp8 DoubleRow over both m-chunks at
           once (ones column gives softmax row sums r); then on DVE:
           1/r, U*(1/r), + static AV read straight from PSUM -> ob.
  tr(b-1): O^T via PE transpose -> aot (proj lhsT layout).
  st_av(b): S^T = k q^T (bf16) -> exp(S/8 - 2) on ACT -> fp8 P^T
           (the -2 keeps exp below the TRN e4m3 max of 240; softmax is
           shift-invariant). A@v (bf16) interleaved between exp-paced
           S^T PSUM groups to keep the PE array busy.
  proj   : out = aot.T @ W_proj + b_proj   bf16, bias via DVE add.

P^T and v8 live in ping-pong persistent tiles: partitions 68-127 of the
second m-chunk are zeroed once at init so the DoubleRow contraction over
m=196 (padded to 2x128) is exact.
"""

import os
import sys

_TRN_REPO = "/opt/trn_rl_repo"
if _TRN_REPO not in sys.path:
    sys.path.insert(0, _TRN_REPO)

import numpy as np
import ml_dtypes

import concourse.bass as bass
import concourse.tile as tile
from concourse import bacc, mybir
from concourse.bass import MemorySpace
from concourse.bass_utils import run_bass_kernel_spmd
from concourse.masks import make_identity

BF16 = mybir.dt.bfloat16
F32 = mybir.dt.float32
FP8 = mybir.dt.float8e4

N_CORES = 8
BATCH = 64
B = BATCH // N_CORES  # 8 local batches per core
H = 12
D = 64
N = 196
C = 768
T = B * N  # 1568 local tokens
KC = 6  # contraction chunks of 128 over C=768
G = 3  # DoubleRow contraction groups of 256 over C=768
SCALE = D ** -0.5  # 0.125
ESHIFT = -2.0  # exp(S/8 - 2): keeps P under the TRN e4m3 max (240)

# token chunks of 128 over T (for the projection)
MCS = [(i * 128, min(128, T - i * 128)) for i in range((T + 127) // 128)]
# per-batch row chunks over N=196
NCH = [(0, 128), (128, 68)]

AluOp = mybir.AluOpType
ActFn = mybir.ActivationFunctionType
PM = mybir.MatmulPerfMode


def _emit(nc: bass.Bass):
    # xt: per-batch partition-major x^T blocks: xt[b, p, kc*N+n] = x[b, n, kc*128+p]
    # x8: same layout in fp8 (for the q/k DoubleRow GEMM)
    # at: partition-major A^T: at[mc, p, h*N+n] = A[h, n, mc*128+p]
    xt_d = nc.declare_dram_parameter("xt", [B, 128, KC * N], BF16, isOutput=False)
    x8_d = nc.declare_dram_parameter("x8", [B, 128, KC * N], FP8, isOutput=False)
    wqk8_d = nc.declare_dram_parameter("wqk8", [128, KC * 2 * C], FP8, isOutput=False)
    wv_d = nc.declare_dram_parameter("wv", [128, KC * C], BF16, isOutput=False)
    at_d = nc.declare_dram_parameter("at", [2, 128, H * N], BF16, isOutput=False)
    wproj_d = nc.declare_dram_parameter("wproj", [C, C], BF16, isOutput=False)
    bproj_d = nc.declare_dram_parameter("bproj", [1, C], BF16, isOutput=False)
    out_d = nc.declare_dram_parameter("out", [T, C], F32, isOutput=True)

    with tile.TileContext(nc) as tc:
        from contextlib import ExitStack

        with ExitStack() as stk:
            const = stk.enter_context(tc.tile_pool(name="const", bufs=1))
            wq = stk.enter_context(tc.tile_pool(name="wq", bufs=1))
            xtp = stk.enter_context(tc.tile_pool(name="xtp", bufs=4))
            x8p = stk.enter_context(tc.tile_pool(name="x8p", bufs=4))
            qkp = stk.enter_context(tc.tile_pool(name="qkp", bufs=4))
            vbp = stk.enter_context(tc.tile_pool(name="vbp", bufs=4))
            obp = stk.enter_context(tc.tile_pool(name="obp", bufs=4))
            persist = stk.enter_context(tc.tile_pool(name="persist", bufs=1))
            stage = stk.enter_context(tc.tile_pool(name="stage", bufs=3))
            small = stk.enter_context(tc.tile_pool(name="small", bufs=3))
            outst = stk.enter_context(tc.tile_pool(name="outst", bufs=4))
            aotp = stk.enter_context(tc.tile_pool(name="aotp", bufs=1))

            # ---- constants (cheap; loaded early) ----
            ident = const.tile([128, 128], BF16)
            make_identity(nc, ident)
            cbias = const.tile([128, 1], F32)
            nc.vector.memset(cbias, ESHIFT)

            wqk8_sb = wq.tile([128, KC, 2 * C], FP8)
            wv_sb = wq.tile([128, KC, C], BF16)
            at_sb = const.tile([128, 2, H * N], BF16)
            wproj_sb = const.tile([128, KC, C], BF16)
            bias_bc = const.tile([128, C], BF16)
            aot_sb = aotp.tile([128, KC, T], BF16)

            # ping-pong persistent tiles: P^T (fp8) and v|1 (fp8), with the
            # m-chunk index as a free dim so one DoubleRow matmul contracts
            # all of m=196 (partitions 68-127 of chunk 1 stay zero).
            pt_pp = [
                persist.tile([128, 2, H, N], FP8, name=f"pt{i}", tag=f"pt{i}")
                for i in range(2)
            ]
            v8_pp = [
                persist.tile([128, 2, H, 65], FP8, name=f"v8{i}", tag=f"v8{i}")
                for i in range(2)
            ]
            def emit_pad_memsets():
                # engines need 0/32/64/96-aligned start partitions: zero
                # from 64 (64-67 get valid data every batch).  Emitted after
                # st_av(0) — only uo(0), in the b=1 section, reads the pads,
                # and the DVE is idle during st_av.
                for ptile in pt_pp:
                    nc.vector.memset(ptile[64:128, 1, :, :], 0.0)
                for vtile in v8_pp:
                    nc.vector.memset(vtile[64:128, 1, :, :], 0.0)
                    nc.vector.memset(vtile[0:128, 0, :, 64:65], 1.0)
                    nc.vector.memset(vtile[0:64, 1, :, 64:65], 1.0)
                    nc.vector.memset(vtile[64:68, 1, :, 64:65], 1.0)

            # per-batch rotating tiles, tracked across loop iterations
            xtb_t = {}
            x8b_t = {}
            qtb_t = {}
            ktb_t = {}
            vb_t = {}
            ob_t = {}

            def emit_load_x(b):
                xtb = xtp.tile([128, KC, N], BF16, tag="xtb")
                x8b = x8p.tile([128, KC, N], FP8, tag="x8b")
                xtb_t[b], x8b_t[b] = xtb, x8b
                nc.sync.dma_start(out=xtb.rearrange("p k n -> p (k n)"), in_=xt_d[b])
                # Pool-issued DMA: keeps the SP sequencer free (a DMA holds
                # its issuing engine's SEQ for the whole transfer)
                nc.gpsimd.dma_start(
                    out=x8b.rearrange("p k n -> p (k n)"), in_=x8_d[b]
                )

            def emit_qk(b):
                # q^T, k^T via fp8 DoubleRow: lhsT = W slice [128, 2, 128]
                # (256-deep contraction per group), rhs = x^T [128, 2, 196].
                x8b = x8b_t[b]
                qtb = qkp.tile([128, KC, N], BF16, tag="qtb")
                ktb = qkp.tile([128, KC, N], BF16, tag="ktb")
                qtb_t[b], ktb_t[b] = qtb, ktb
                stq = stage.tile([64, KC, N], BF16, tag="stq")
                stk_ = stage.tile([64, KC, N], BF16, tag="stk")
                for si, (dst, sec, stg) in enumerate(
                    ((qtb, 0, stq), (ktb, C, stk_))
                ):
                    for cp in range(3):  # pairs of 128-col chunks
                        ps = psA.tile([128, 512], F32, tag="pA")
                        for cc in range(2):
                            c = cp * 2 + cc
                            for g in range(G):
                                nc.tensor.matmul(
                                    ps[:, cc * 256 : cc * 256 + N],
                                    lhsT=wqk8_sb[
                                        :, 2 * g : 2 * g + 2,
                                        sec + c * 128 : sec + (c + 1) * 128,
                                    ],
                                    rhs=x8b[:, 2 * g : 2 * g + 2, :],
                                    start=(g == 0),
                                    stop=(g == G - 1),
                                    perf_mode=PM.DoubleRow,
                                )
                        dstv = dst[:, cp * 2 : cp * 2 + 2, :]
                        srcv = ps.rearrange("p (c n) -> p c n", c=2)[:, :, 0:N]
                        # alternate engines so copies pace at ~2x
                        if (si * 3 + cp) % 2 == 0:
                            nc.vector.tensor_copy(dstv, srcv)
                        else:
                            nc.scalar.copy(dstv, srcv)
                    nc.gpsimd.dma_start(out=stg, in_=dst[64:128, :, :])
                return stq, stk_

            def emit_v_quarter(b, mc, ns):
                xtb = xtb_t[b]
                v8t = v8_pp[b % 2]
                if b not in vb_t:
                    vb_t[b] = vbp.tile([128, 2, H * D], BF16, name="vb", tag="vb")
                vb = vb_t[b]
                moff, mlen = NCH[mc]
                ps = psA.tile([128, 512], F32, tag="pA")
                for kc in range(KC):
                    nc.tensor.matmul(
                        ps[0:mlen, 0:384],
                        lhsT=xtb[:, kc, moff : moff + mlen],
                        rhs=wv_sb[:, kc, ns * 384 : (ns + 1) * 384],
                        start=(kc == 0),
                        stop=(kc == KC - 1),
                    )
                dstv = vb[0:mlen, mc, ns * 384 : (ns + 1) * 384]
                if mc == 1 and ns == 1:
                    nc.vector.tensor_copy(dstv, ps[0:mlen, 0:384])
                else:
                    nc.scalar.copy(dstv, ps[0:mlen, 0:384])
                # fp8 copy for the U matmul (Pool: SBUF->SBUF only)
                nc.gpsimd.tensor_copy(
                    v8t[0:mlen, mc, ns * 6 : (ns + 1) * 6, 0:64],
                    dstv.rearrange("p (h c) -> p h c", h=6),
                )

            av_t = {}

            def emit_st_av(b, stq, stk_, fillers=()):
                # Interleave the A@v matmul blocks between the exp-paced S^T
                # PSUM groups: PE executes in order, so a stalled S^T group
                # would otherwise idle the array while ACT drains exp.
                qtb, ktb = qtb_t[b], ktb_t[b]
                vb = vb_t[b]
                ptt = pt_pp[b % 2]

                def st_unit(mc, hg):
                    moff, mlen = NCH[mc]
                    ps = psB.tile([128, 1024], F32, tag="pB")
                    for hh in range(4):
                        h = hg * 4 + hh
                        off = (hh // 2) * 512 + (hh % 2) * 196
                        if h % 2 == 0:
                            lhsT = ktb[0:64, h // 2, moff : moff + mlen]
                            rhs = qtb[0:64, h // 2, :]
                        else:
                            lhsT = stk_[0:64, h // 2, moff : moff + mlen]
                            rhs = stq[0:64, h // 2, :]
                        nc.tensor.matmul(
                            ps[0:mlen, off : off + 196],
                            lhsT=lhsT,
                            rhs=rhs,
                            start=True,
                            stop=True,
                        )
                    src = ps.rearrange("p (k x) -> p k x", k=2)[
                        0:mlen, :, 0:392
                    ].rearrange("p k (h n) -> p k h n", h=2)
                    dst = ptt[0:mlen, mc, hg * 4 : (hg + 1) * 4, :].rearrange(
                        "p (k h) n -> p k h n", k=2
                    )
                    nc.scalar.activation(
                        dst, src, ActFn.Exp, bias=cbias[0:mlen, :], scale=SCALE
                    )

                av_tile = {}

                def av_unit(nc_i, hblk):
                    noff, nlen = NCH[nc_i]
                    if nc_i not in av_tile:
                        av = psB.tile([128, 1024], F32, tag="pB")
                        av_tile[nc_i] = av
                        av_t[(b, nc_i)] = av
                    av = av_tile[nc_i]
                    for h in range(hblk * 3, hblk * 3 + 3):
                        aoff = (h // 8) * 512 + (h % 8) * 64
                        for mc, (moff, mlen) in enumerate(NCH):
                            nc.tensor.matmul(
                                av[0:nlen, aoff : aoff + 64],
                                lhsT=at_sb[
                                    0:mlen, mc, h * N + noff : h * N + noff + nlen
                                ],
                                rhs=vb[0:mlen, mc, h * D : h * D + D],
                                start=(mc == 0),
                                stop=(mc == 1),
                            )

                order = [
                    ("st", 0, 0), ("av", 0, 0), ("st", 0, 1), ("av", 0, 1),
                    ("st", 0, 2), ("av", 0, 2), ("st", 1, 0), ("av", 0, 3),
                    ("st", 1, 1), ("av", 1, 0), ("st", 1, 2), ("av", 1, 1),
                    ("av", 1, 2), ("av", 1, 3),
                ]
                fi = 0
                for kind, a, bb_ in order:
                    if kind == "st":
                        st_unit(a, bb_)
                    else:
                        av_unit(a, bb_)
                    # weave previous-batch epilogue work (adds, transposes,
                    # ready proj chunks) between the exp-paced S^T groups:
                    # PE filler + deeper psB rotation spacing
                    if fi < len(fillers):
                        fillers[fi]()
                        fi += 1
                while fi < len(fillers):
                    fillers[fi]()
                    fi += 1

            tmp_t = {}

            def emit_u_unit(b, nc_i, half):
                # U = P^T.T @ [v|1] via one fp8 DoubleRow matmul per head
                # (both m-chunks contracted at once); 65th column is the
                # softmax row sum r.  tmp = U * (1/r) on DVE.
                ptt = pt_pp[b % 2]
                v8t = v8_pp[b % 2]
                noff, nlen = NCH[nc_i]
                if (b, nc_i) not in tmp_t:
                    rec = small.tile([128, H], F32, tag="rec")
                    tmp = small.tile([128, C], F32, tag="tmp")
                    tmp_t[(b, nc_i)] = (rec, tmp)
                rec, tmp = tmp_t[(b, nc_i)]
                uph = psA.tile([128, 512], F32, tag="pA")
                for h in range(half * 6, half * 6 + 6):
                    uoff = (h % 6) * 65
                    nc.tensor.matmul(
                        uph[0:nlen, uoff : uoff + 65],
                        lhsT=ptt[0:128, 0:2, h, noff : noff + nlen],
                        rhs=v8t[0:128, 0:2, h, 0:65],
                        start=True,
                        stop=True,
                        perf_mode=PM.DoubleRow,
                    )
                upv = uph[0:nlen, 0:390].rearrange("p (h x) -> p h x", h=6)
                recv = rec[0:nlen, half * 6 : half * 6 + 6, None]
                nc.vector.reciprocal(recv, upv[:, :, 64:65])
                nc.vector.tensor_tensor(
                    tmp[0:nlen, half * 384 : (half + 1) * 384].rearrange(
                        "p (h c) -> p h c", h=6
                    ),
                    upv[:, :, 0:64],
                    recv.to_broadcast((nlen, 6, 64)),
                    AluOp.mult,
                )

            def emit_u_add(b, nc_i, half):
                # ob = tmp + AV, with AV read straight from its PSUM tile
                noff, nlen = NCH[nc_i]
                if b not in ob_t:
                    ob_t[b] = obp.tile([128, 2, C], BF16, name="ob", tag="ob")
                ob = ob_t[b]
                av = av_t[(b, nc_i)]
                _, tmp = tmp_t[(b, nc_i)]
                nc.vector.tensor_tensor(
                    ob[0:nlen, nc_i, half * 384 : (half + 1) * 384],
                    tmp[0:nlen, half * 384 : (half + 1) * 384],
                    av[0:nlen, half * 384 : (half + 1) * 384],
                    AluOp.add,
                )

            def emit_tr_pair(b, nc_i, hp):
                ob = ob_t[b]
                noff, nlen = NCH[nc_i]
                tp = psB.tile([128, 512], BF16, tag="pB")
                for j in range(2):
                    nc.tensor.transpose(
                        tp[:, j * 256 : j * 256 + nlen],
                        in_=ob[0:nlen, nc_i, (hp + j) * 128 : (hp + j + 1) * 128],
                        identity=ident[0:nlen, 0:nlen],
                    )
                nc.vector.tensor_copy(
                    aot_sb[:, hp : hp + 2, b * N + noff : b * N + noff + nlen],
                    tp.rearrange("p (j x) -> p j x", j=2)[:, :, 0:nlen],
                )

            def emit_proj_chunk(mc, pps, tag="pB"):
                moff, mlen = MCS[mc]
                pp = pps.tile([128, 1024], F32, tag=tag)
                for nsl, nw in ((0, 512), (512, 256)):
                    for kc in range(KC):
                        nc.tensor.matmul(
                            pp[0:mlen, nsl : nsl + nw],
                            lhsT=aot_sb[:, kc, moff : moff + mlen],
                            rhs=wproj_sb[:, kc, nsl : nsl + nw],
                            start=(kc == 0),
                            stop=(kc == KC - 1),
                        )
                ot = outst.tile([128, C], F32, tag="ot")
                nc.vector.tensor_tensor(
                    ot[0:mlen, :],
                    pp[0:mlen, 0:768],
                    bias_bc[0:mlen, :],
                    AluOp.add,
                )
                nc.sync.dma_start(
                    out=out_d[moff : moff + mlen, :], in_=ot[0:mlen, :]
                )

            with (
                tc.tile_pool(name="psA", bufs=2, space=MemorySpace.PSUM) as psA,
                tc.tile_pool(name="psB", bufs=3, space=MemorySpace.PSUM) as psB,
            ):
                # input DMAs for batch 0; weights spread across the SP/ACT/
                # Pool queues (a DMA occupies its issuing engine's sequencer
                # for the whole transfer, so one queue would serialize them)
                nc.sync.dma_start(
                    out=wqk8_sb[:, 0:3, :], in_=wqk8_d[:, 0 : 3 * 2 * C]
                )
                nc.scalar.dma_start(
                    out=wqk8_sb[:, 3:6, :], in_=wqk8_d[:, 3 * 2 * C : 6 * 2 * C]
                )
                emit_load_x(0)
                nc.scalar.dma_start(out=wv_sb[:, 0:3, :], in_=wv_d[:, 0 : 3 * C])
                nc.gpsimd.dma_start(
                    out=wv_sb[:, 3:6, :], in_=wv_d[:, 3 * C : 6 * C]
                )
                for mc in range(2):
                    nc.sync.dma_start(out=at_sb[:, mc, :], in_=at_d[mc])
                def uo_tr_fillers(b, chunks):
                    # epilogue units for batch b, in dependency order:
                    # tr(nc,0) needs add(nc,0); tr(nc,1) both; tr(nc,2)
                    # needs add(nc,1); proj chunks need all trs of batch b
                    fs = []
                    for nc_i in range(2):
                        fs.append(lambda n=nc_i: emit_u_add(b, n, 0))
                        fs.append(lambda n=nc_i: emit_u_add(b, n, 1))
                        for hp in range(0, KC, 2):
                            fs.append(lambda n=nc_i, p=hp: emit_tr_pair(b, n, p))
                    for i in chunks:
                        fs.append(lambda i=i: emit_proj_chunk(i, psB, tag="pB"))
                    return fs

                # proj chunk i covers tokens [128i, 128(i+1)): ready once the
                # last batch it touches has been transposed into aot
                chunk_ready = {}
                for i in range(len(MCS)):
                    last_b = (min((i + 1) * 128, T) - 1) // N
                    chunk_ready.setdefault(last_b, []).append(i)

                stqk = {}
                for b in range(B):
                    stqk[b] = emit_qk(b)
                    if b + 1 < B:
                        emit_load_x(b + 1)
                    # v quarters interleaved with U units of the previous
                    # batch: alternating psA consumers (ACT/Pool vs DVE)
                    # so neither engine paces the pool rotation alone
                    quarters = [(0, 0), (0, 1), (1, 0), (1, 1)]
                    for qi, (mc, ns) in enumerate(quarters):
                        emit_v_quarter(b, mc, ns)
                        if b > 0:
                            emit_u_unit(b - 1, qi // 2, qi % 2)
                    fillers = (
                        uo_tr_fillers(b - 1, chunk_ready.get(b - 1, []))
                        if b > 0
                        else []
                    )
                    emit_st_av(b, *stqk[b], fillers=fillers)
                    if b == 0:
                        emit_pad_memsets()
                        # wproj in two 3-chunk DMAs (3D source AP)
                        for half in range(2):
                            wp_ap = bass.AP(
                                tensor=wproj_d.ap().tensor,
                                offset=half * 3 * 128 * C,
                                ap=[[C, 128], [128 * C, 3], [1, C]],
                            )
                            nc.sync.dma_start(
                                out=wproj_sb[:, half * 3 : half * 3 + 3, :],
                                in_=wp_ap,
                            )
                        bproj_ap = bass.AP(
                            tensor=bproj_d.ap().tensor,
                            offset=0,
                            ap=[[0, 128], [1, C]],
                        )
                        nc.gpsimd.dma_start(out=bias_bc, in_=bproj_ap)
                # tail: batch B-1 epilogue with the last proj chunks woven in
                # as PE filler (chunk 10 only needs the nc_i=0 transposes)
                last_chunks = chunk_ready.get(B - 1, [])
                for qi in range(4):
                    emit_u_unit(B - 1, qi // 2, qi % 2)
                bL = B - 1
                emit_u_add(bL, 0, 0)
                emit_u_add(bL, 0, 1)
                for hp in range(0, KC, 2):
                    emit_tr_pair(bL, 0, hp)
                if last_chunks:
                    emit_proj_chunk(last_chunks[0], psB, tag="pB")
                emit_u_add(bL, 1, 0)
                emit_u_add(bL, 1, 1)
                for hp in range(0, KC, 2):
                    emit_tr_pair(bL, 1, hp)
                for i in last_chunks[1:]:
                    emit_proj_chunk(i, psB, tag="pB")

    return nc


_CACHE: dict = {}


def _get_module():
    if "nc" not in _CACHE:
        nc = bacc.Bacc(None, target_bir_lowering=False)
        _emit(nc)
        nc.compile()
        _CACHE["nc"] = nc
    return _CACHE["nc"]


def _prep_inputs(x, W_qkv, static_a, W_proj, b_proj):
    """Host-side shard + layout prep; returns per-core input maps."""
    bf = ml_dtypes.bfloat16
    f8 = ml_dtypes.float8_e4m3
    x = np.asarray(x, dtype=np.float32)
    Wf = np.asarray(W_qkv, dtype=np.float32)
    # kc-major weight views: [128, KC, cols]
    Wr = Wf.reshape(KC, 128, 3 * C)
    wqk8_b = np.ascontiguousarray(
        Wr[:, :, 0 : 2 * C].transpose(1, 0, 2)
    ).reshape(128, KC * 2 * C).astype(f8)
    wv_b = np.ascontiguousarray(
        Wr[:, :, 2 * C : 3 * C].transpose(1, 0, 2)
    ).reshape(128, KC * C).astype(bf)
    A = np.asarray(static_a, dtype=np.float32)[0]  # [H, N, N]
    Am = np.ascontiguousarray(A.transpose(2, 0, 1))  # [m, H, n]
    at_arr = np.zeros((2, 128, H, N), dtype=np.float32)
    at_arr[0] = Am[0:128]
    at_arr[1, 0:68] = Am[128:196]
    at_b = at_arr.reshape(2, 128, H * N).astype(bf)
    wproj_b = np.asarray(W_proj, dtype=np.float32).astype(bf)
    bproj_b = np.asarray(b_proj, dtype=np.float32).reshape(1, C).astype(bf)

    in_maps = []
    for i in range(N_CORES):
        shard = x[i * B : (i + 1) * B]  # [B, N, C]
        # [B, 128, KC*N]: xt[b, p, kc*N + n] = x[b, n, kc*128 + p]
        xt_f = np.ascontiguousarray(
            shard.transpose(0, 2, 1)
            .reshape(B, KC, 128, N)
            .transpose(0, 2, 1, 3)
            .reshape(B, 128, KC * N)
        )
        in_maps.append(
            dict(
                xt=xt_f.astype(bf),
                x8=xt_f.astype(f8),
                wqk8=wqk8_b,
                wv=wv_b,
                at=at_b,
                wproj=wproj_b,
                bproj=bproj_b,
            )
        )
    return in_maps


_last_results = None


def kernel(x, W_qkv, static_a, W_proj, b_proj):
    global _last_results
    in_maps = _prep_inputs(x, W_qkv, static_a, W_proj, b_proj)
    nc = _get_module()
    res = run_bass_kernel_spmd(nc, in_maps, core_ids=list(range(N_CORES)))
    _last_results = res
    out = np.concatenate(
        [np.asarray(r["out"]).reshape(B, N, C) for r in res.results], axis=0
    )
    return out.astype(np.float32)
